# revision 34
# baseline (speedup 1.0000x reference)
"""Trainium2 Bass kernel for the deterministic legality module.

Computes, for each board b, filter f and top-left placement (i,j):
    legal[b,f,i,j] = 1.0 iff every occupied cell of filter f, placed at
    (i,j), lands in-bounds on a free cell of board b (and f is non-empty).

Structure exploited (all derived from the actual filter data at
kernel-build time, so the kernel stays correct for any filter set):

  * A placement (i,j) of filter f with max tap offsets (mdy, mdx) is
    structurally illegal unless i <= 8-mdy and j <= 8-mdx; only ~1/3 of
    the 264*81 output columns are reachable.  The device computes ONLY
    those C columns; the host scatters into the zeroed full output.
  * Duplicate filter patterns share one device column.
  * legal = relu(corr + thr) exactly, with the threshold rows folded
    into the contraction (board side carries ones rows).
  * The 0/1 result goes to HBM as int8 (4x less write traffic).

The matmul runs in fp8e4 DoubleRow mode: K = 84 as two k-tiles of 42
(k = q*42+p), which streams two contraction rows per cycle -> 0.5
cycles per output column, measured ~213ns per 512-column tile.
k 0..80 = board cells, k 81/82 = two threshold rows (thr = 1-area split
ceil/floor so every value is e4m3-exact), k 83 = zero pad.  M
([42, 2, C] fp8) is built ON THE HOST and DMA'd in; there is no
on-device build phase.

Sharding: pure data parallelism, batch 4096 -> 512 per core on 8 cores.
"""

import numpy as np
import ml_dtypes

N_CORES = 8
BATCH = 4096
BPC = BATCH // N_CORES  # 512 boards per core
NPOS = 81               # 9x9 board cells / placements
NF = 264                # filters
KT = 42                 # k-tile size (DoubleRow)
K = 2 * KT              # logical contraction: 81 cells + 2 thr + pad

COL_TILE = 512          # one PSUM bank of f32
COL_GROUP = 1024        # columns per drain / output DMA chunk
WARMUP_MM = 2           # dummy matmuls bridging input-DMA latency


def _plan(filters):
    """Host-side column plan + DoubleRow M matrix from the filter data.

    Returns (M[KT, 2*c_pad] fp8, c_pad, f_sc, ij_sc, c_sc) where the
    scatter triplet satisfies full[:, f_sc, ij_sc] = raw[:, c_sc].
    M[p, q*c_pad + c] holds logical row k = q*42+p: taps for k<=80,
    thr_a at 81, thr_b at 82, zero at 83.
    """
    filt = np.asarray(filters, dtype=np.float32).reshape(NF, 5, 5)
    areas = filt.sum(axis=(1, 2))
    occ = filt > 0.5

    nonempty = np.where(areas > 0.5)[0]
    keys = (occ.reshape(NF, 25).astype(np.int64)
            * (1 << np.arange(25, dtype=np.int64))).sum(axis=1)
    _, first, inv = np.unique(keys[nonempty], return_index=True,
                              return_inverse=True)
    reps = nonempty[first]          # representative filter per pattern
    U = len(reps)

    mdy = np.array([occ[r].any(axis=1).nonzero()[0].max() for r in reps])
    mdx = np.array([occ[r].any(axis=0).nonzero()[0].max() for r in reps])

    cols = []                       # (u, i, j), ij-major
    col_of = {}                     # (u, ij) -> c
    for i in range(9):
        for j in range(9):
            for u in range(U):
                if mdy[u] <= 8 - i and mdx[u] <= 8 - j:
                    col_of[(u, i * 9 + j)] = len(cols)
                    cols.append((u, i, j))
    C = len(cols)
    c_pad = -(-C // COL_GROUP) * COL_GROUP   # even # of 512-tiles (A/B split)

    M82 = np.zeros((K, c_pad), dtype=np.float32)
    for c, (u, i, j) in enumerate(cols):
        r = reps[u]
        dys, dxs = np.nonzero(occ[r])
        M82[(i + dys) * 9 + (j + dxs), c] = 1.0
        thr = 1.0 - areas[r]
        M82[NPOS, c] = np.ceil(thr / 2)       # thr_a, in [-12, 0]
        M82[NPOS + 1, c] = np.floor(thr / 2)  # thr_b, in [-12, 0]

    # interleave into DoubleRow k-tiles: M3[p, q, c] = M82[q*42+p, c],
    # then split columns: even 512-tiles -> A (PE row strips 0-1), odd
    # 512-tiles -> B (strips 2-3) so consecutive matmuls alternate
    # row-groups and their weight loads hide under each other.  Layout
    # is tile-major ([KT, tile, q, 512]) so each tile is a contiguous
    # 1KB-per-partition DMA chunk.
    M3 = M82.reshape(2, KT, c_pad).transpose(1, 0, 2)   # [KT, 2, c_pad]
    t = M3.reshape(KT, 2, c_pad // COL_TILE, COL_TILE).transpose(0, 2, 1, 3)
    MA = np.ascontiguousarray(t[:, 0::2].reshape(KT, -1))
    MB = np.ascontiguousarray(t[:, 1::2].reshape(KT, -1))

    f_sc, ij_sc, c_sc = [], [], []
    for fi, f in enumerate(nonempty):
        u = inv[fi]
        for ij in range(NPOS):
            c = col_of.get((u, ij))
            if c is not None:
                f_sc.append(f)
                ij_sc.append(ij)
                c_sc.append(c)
    return (MA.astype(ml_dtypes.float8_e4m3fn),
            MB.astype(ml_dtypes.float8_e4m3fn), c_pad,
            np.asarray(f_sc), np.asarray(ij_sc), np.asarray(c_sc))


def _build_module(c_pad):
    import concourse.bass as bass
    import concourse.mybir as mybir
    import concourse.tile as tile
    from concourse.masks import make_identity

    f32 = mybir.dt.float32
    fp8 = mybir.dt.float8e4
    i8 = mybir.dt.int8
    Relu = mybir.ActivationFunctionType.Relu
    DR = mybir.MatmulPerfMode.DoubleRow

    nc = bass.Bass("TRN2", target_bir_lowering=False, debug=False,
                   num_devices=N_CORES)

    board_d = nc.dram_tensor("board", [KT, 2 * BPC], fp8,
                             kind="ExternalInput")
    c_half = c_pad // 2
    ma_d = nc.dram_tensor("mmatA", [KT, 2 * c_half], fp8, kind="ExternalInput")
    mb_d = nc.dram_tensor("mmatB", [KT, 2 * c_half], fp8, kind="ExternalInput")
    out_d = nc.dram_tensor("out", [BPC, c_pad], i8, kind="ExternalOutput")

    groups = [(g, min(g + COL_GROUP, c_pad))
              for g in range(0, c_pad, COL_GROUP)]
    nkb = BPC // 128
    PB = 64                    # partition base of the B row-strip copy

    with tile.TileContext(nc) as tc:
        with tc.tile_pool(name="const", bufs=1) as cpool:
            ident = cpool.tile([128, 128], f32)
            make_identity(nc, ident[:])

            # A copies live at partitions 0..41 (PE row strips 0-1), B
            # copies at 64..105 (strips 2-3): consecutive matmuls
            # alternate strips so LDWEIGHTS overlaps the other strip's
            # matmul instead of serializing with it.
            ntile = c_half // COL_TILE
            Mbig = cpool.tile([PB + KT, 2 * c_half], fp8)
            MA3 = Mbig[0:KT, :].rearrange("p (t q n) -> p t q n", t=ntile,
                                          q=2)
            MB3 = Mbig[PB:PB + KT, :].rearrange("p (t q n) -> p t q n",
                                                t=ntile, q=2)
            bbig = cpool.tile([PB + KT, 2 * BPC], fp8)
            bA3 = bbig[0:KT, :].rearrange("p (q n) -> p q n", q=2)
            bB3 = bbig[PB:PB + KT, :].rearrange("p (q n) -> p q n", q=2)

            # ---- phase A: inputs + act-table preload -------------------
            # The board arrives pre-transposed from the host ([KT, 2,
            # 512] fp8, 43KB), so there is no on-device transpose work
            # at all: both PE row-strip copies load straight from HBM.
            with (
                tc.tile_pool(name="prep", bufs=2) as prep,
                tc.tile_pool(name="psW", bufs=1, space="PSUM") as psW,
            ):
                def m_load(eng, which, gi0, gi1):
                    s0, s1 = gi0 * 2 * COL_TILE, gi1 * 2 * COL_TILE
                    if which == 0:
                        eng.dma_start(Mbig[0:KT, s0:s1], ma_d[:, s0:s1])
                    else:
                        eng.dma_start(Mbig[PB:PB + KT, s0:s1],
                                      mb_d[:, s0:s1])

                ngr = len(groups)
                nc.sync.dma_start(bbig[0:KT, :], board_d[:])
                m_load(nc.sync, 0, 0, 1)
                nc.sync.dma_start(bbig[PB:PB + KT, :], board_d[:])
                m_load(nc.sync, 1, 0, 1)
                m_load(nc.sync, 0, 1, 2)
                m_load(nc.sync, 0, 2, 3)
                m_load(nc.sync, 0, 3, 5)
                m_load(nc.sync, 0, 5, ngr)
                m_load(nc.sync, 1, 3, 5)
                m_load(nc.sync, 1, 5, ngr)

                # preload the Relu activation table on an independent
                # dummy read of ident, then use ACT's idle early window
                # for the early B-strip M loads (the late ones ride SP).
                warm8 = prep.tile([1, 2], i8, tag="warm8")
                nc.scalar.activation(warm8[:], ident[0:1, 0:2], Relu)
                m_load(nc.scalar, 1, 1, 2)
                m_load(nc.scalar, 1, 2, 3)

                if WARMUP_MM:
                    wps0 = psW.tile([128, 128], f32, tag="warm")
                    for _ in range(WARMUP_MM):
                        nc.tensor.matmul(wps0[:], ident[:], ident[:],
                                         start=True, stop=True)
                    wrd = prep.tile([32, 1], f32, tag="wrd")
                    nc.vector.tensor_scalar_add(wrd[:], wps0[0:32, 0:1], 0.0)

            # ---- phase B: DoubleRow matmul + relu(int8) + store --------
            # group-outer / kb-inner.  Drains alternate DVE/ACT except
            # the final group (all DVE, so ACT is free to co-trigger the
            # final output wave).  Output chunks: first three groups on
            # the software DGE, middle on SP, final wave as half-height
            # chunks on SP+ACT for a short tail.
            nchunks = len(groups) * nkb
            with (
                tc.tile_pool(name="psM", bufs=4, space="PSUM") as psM,
                tc.tile_pool(name="ostage", bufs=nchunks) as ostage,
            ):
                # ostage has one buffer per chunk: drains never wait on
                # output DMAs and the DMA triggers can lag freely.
                last_g0 = groups[-1][0]
                ci = 0
                for gi, (g0, g1) in enumerate(groups):
                    final = g0 == last_g0
                    for kb in range(nkb):
                        ks = slice(kb * 128, (kb + 1) * 128)
                        pt = psM.tile([128, COL_GROUP], f32, tag="mm")
                        # even 512-tile from strip A, odd from strip B;
                        # their LDWEIGHTS overlap each other's matmuls
                        nc.tensor.matmul(
                            pt[:, 0:COL_TILE], bA3[:, :, ks],
                            MA3[:, gi, :, :],
                            start=True, stop=True, perf_mode=DR,
                            tile_position=(0, 0))
                        nc.tensor.matmul(
                            pt[:, COL_TILE:COL_GROUP], bB3[:, :, ks],
                            MB3[:, gi, :, :],
                            start=True, stop=True, perf_mode=DR,
                            tile_position=(PB, 0))
                        ot = ostage.tile([128, COL_GROUP], i8, tag="ot")
                        # both engines drain one half each in parallel:
                        # half the per-chunk drain latency, so the
                        # 4-deep PSUM pipeline stops hiccuping at pool
                        # wrap-around.  ACT (the busier engine) takes
                        # the A half, which is ready a matmul earlier.
                        nc.scalar.activation(
                            ot[:, 0:COL_TILE], pt[:, 0:COL_TILE], Relu)
                        nc.vector.tensor_scalar_max(
                            ot[:, COL_TILE:COL_GROUP],
                            pt[:, COL_TILE:COL_GROUP], 0.0)
                        ci += 1
                        rows = out_d[kb * 128:(kb + 1) * 128, g0:g1]
                        if final:
                            nc.sync.dma_start(rows[0:64, :], ot[0:64, :])
                            nc.sync.dma_start(rows[64:128, :],
                                              ot[64:128, :])
                        elif gi < 3 or ci % 2 == 0:
                            nc.gpsimd.dma_start(rows, ot[:])
                        else:
                            nc.sync.dma_start(rows, ot[:])
    return nc


def _legalize_multiwait(nc):
    """Split multi-wait instructions for this walrus build.

    The TPB instruction encodings carry exactly one semaphore wait, and
    the walrus codegen here refuses instructions with more ("Too many
    sync wait commands").  Hoist all but one wait onto EventSemaphore
    carrier instructions placed immediately before, on the same engine —
    the sequencer blocks on each carrier first, which is semantically
    identical.
    """
    import concourse.mybir as mybir

    for func in nc.m.functions:
        for blk in func.blocks:
            out = []
            changed = False
            for inst in blk.instructions:
                si = inst.sync_info
                waits = list(si.on_wait) if si is not None and si.on_wait else []
                if len(waits) > 1:
                    for j, w in enumerate(waits[:-1]):
                        carrier = mybir.InstEventSemaphore(
                            name=f"{inst.name}-xw{j}",
                            engine=inst.engine,
                            ins=[], outs=[],
                            sync_info=mybir.SyncInfo(on_wait=[w],
                                                     on_update=[]),
                        )
                        nc.register_instruction(carrier)
                        out.append(carrier)
                    inst.sync_info = mybir.SyncInfo(
                        on_wait=[waits[-1]],
                        on_update=list(si.on_update) if si.on_update else [])
                    changed = True
                out.append(inst)
            if changed:
                blk.instructions = out


_CACHE = {}


def _get_module(c_pad):
    if c_pad not in _CACHE:
        nc = _build_module(c_pad)
        _legalize_multiwait(nc)
        _CACHE[c_pad] = nc
    return _CACHE[c_pad]


def run(board_free, filters, areas, trace=False, **spmd_kwargs):
    from concourse.bass_utils import run_bass_kernel_spmd

    MA, MB, c_pad, f_sc, ij_sc, c_sc = _plan(filters)

    # pre-transpose the boards on the host into the DoubleRow k-tile
    # layout: bT[p, q, b] = [cells, one, one, zero][q*42+p] for board b
    board = np.asarray(board_free, dtype=np.float32).reshape(BATCH, NPOS)
    bT = np.empty((K, BATCH), dtype=np.float32)
    bT[0:NPOS] = board.T
    bT[NPOS] = 1.0
    bT[NPOS + 1] = 1.0
    bT[NPOS + 2:] = 0.0
    bdr = (bT.reshape(2, KT, N_CORES, BPC).transpose(2, 1, 0, 3)
           .reshape(N_CORES, KT, 2 * BPC).astype(ml_dtypes.float8_e4m3fn))
    bdr = np.ascontiguousarray(bdr)

    in_maps = [
        {"board": bdr[c], "mmatA": MA, "mmatB": MB}
        for c in range(N_CORES)
    ]
    nc = _get_module(c_pad)
    res = run_bass_kernel_spmd(nc, in_maps, core_ids=list(range(N_CORES)),
                               trace=trace, **spmd_kwargs)
    raw = np.concatenate([r["out"] for r in res.results], axis=0)

    full = np.zeros((BATCH, NF, NPOS), dtype=np.float32)
    full[:, f_sc, ij_sc] = raw[:, c_sc]
    return full.reshape(BATCH, NF, 9, 9), res


def kernel(board_free, filters, areas):
    out, _ = run(board_free, filters, areas)
    return out


# revision 37
# speedup vs baseline: 1.0117x; 1.0117x over previous
"""Trainium2 Bass kernel for the deterministic legality module.

Computes, for each board b, filter f and top-left placement (i,j):
    legal[b,f,i,j] = 1.0 iff every occupied cell of filter f, placed at
    (i,j), lands in-bounds on a free cell of board b (and f is non-empty).

Structure exploited (all derived from the actual filter data at
kernel-build time, so the kernel stays correct for any filter set):

  * A placement (i,j) of filter f with max tap offsets (mdy, mdx) is
    structurally illegal unless i <= 8-mdy and j <= 8-mdx; only ~1/3 of
    the 264*81 output columns are reachable.  The device computes ONLY
    those C columns; the host scatters into the zeroed full output.
  * Duplicate filter patterns share one device column.
  * legal = relu(corr + thr) exactly, with the threshold rows folded
    into the contraction (board side carries ones rows).
  * The 0/1 result goes to HBM as int8 (4x less write traffic).

The matmul runs in fp8e4 DoubleRow mode: K = 84 as two k-tiles of 42
(k = q*42+p), which streams two contraction rows per cycle -> 0.5
cycles per output column, measured ~213ns per 512-column tile.
k 0..80 = board cells, k 81/82 = two threshold rows (thr = 1-area split
ceil/floor so every value is e4m3-exact), k 83 = zero pad.  M
([42, 2, C] fp8) is built ON THE HOST and DMA'd in; there is no
on-device build phase.

Sharding: pure data parallelism, batch 4096 -> 512 per core on 8 cores.
"""

import numpy as np
import ml_dtypes

N_CORES = 8
BATCH = 4096
BPC = BATCH // N_CORES  # 512 boards per core
NPOS = 81               # 9x9 board cells / placements
NF = 264                # filters
KT = 42                 # k-tile size (DoubleRow)
K = 2 * KT              # logical contraction: 81 cells + 2 thr + pad

COL_TILE = 512          # one PSUM bank of f32
COL_GROUP = 1024        # columns per drain / output DMA chunk
WARMUP_MM = 2           # dummy matmuls bridging input-DMA latency


def _plan(filters):
    """Host-side column plan + DoubleRow M matrix from the filter data.

    Returns (M[KT, 2*c_pad] fp8, c_pad, f_sc, ij_sc, c_sc) where the
    scatter triplet satisfies full[:, f_sc, ij_sc] = raw[:, c_sc].
    M[p, q*c_pad + c] holds logical row k = q*42+p: taps for k<=80,
    thr_a at 81, thr_b at 82, zero at 83.
    """
    filt = np.asarray(filters, dtype=np.float32).reshape(NF, 5, 5)
    areas = filt.sum(axis=(1, 2))
    occ = filt > 0.5

    nonempty = np.where(areas > 0.5)[0]
    keys = (occ.reshape(NF, 25).astype(np.int64)
            * (1 << np.arange(25, dtype=np.int64))).sum(axis=1)
    _, first, inv = np.unique(keys[nonempty], return_index=True,
                              return_inverse=True)
    reps = nonempty[first]          # representative filter per pattern
    U = len(reps)

    mdy = np.array([occ[r].any(axis=1).nonzero()[0].max() for r in reps])
    mdx = np.array([occ[r].any(axis=0).nonzero()[0].max() for r in reps])

    cols = []                       # (u, i, j), ij-major
    col_of = {}                     # (u, ij) -> c
    for i in range(9):
        for j in range(9):
            for u in range(U):
                if mdy[u] <= 8 - i and mdx[u] <= 8 - j:
                    col_of[(u, i * 9 + j)] = len(cols)
                    cols.append((u, i, j))
    C = len(cols)
    c_pad = -(-C // COL_GROUP) * COL_GROUP   # even # of 512-tiles (A/B split)

    M82 = np.zeros((K, c_pad), dtype=np.float32)
    for c, (u, i, j) in enumerate(cols):
        r = reps[u]
        dys, dxs = np.nonzero(occ[r])
        M82[(i + dys) * 9 + (j + dxs), c] = 1.0
        thr = 1.0 - areas[r]
        M82[NPOS, c] = np.ceil(thr / 2)       # thr_a, in [-12, 0]
        M82[NPOS + 1, c] = np.floor(thr / 2)  # thr_b, in [-12, 0]

    # interleave into DoubleRow k-tiles: M3[p, q, c] = M82[q*42+p, c],
    # then split columns: even 512-tiles -> A (PE row strips 0-1), odd
    # 512-tiles -> B (strips 2-3) so consecutive matmuls alternate
    # row-groups and their weight loads hide under each other.  Layout
    # is tile-major ([KT, tile, q, 512]) so each tile is a contiguous
    # 1KB-per-partition DMA chunk.
    M3 = M82.reshape(2, KT, c_pad).transpose(1, 0, 2)   # [KT, 2, c_pad]
    t = M3.reshape(KT, 2, c_pad // COL_TILE, COL_TILE).transpose(0, 2, 1, 3)
    MA = np.ascontiguousarray(t[:, 0::2].reshape(KT, -1))
    MB = np.ascontiguousarray(t[:, 1::2].reshape(KT, -1))

    f_sc, ij_sc, c_sc = [], [], []
    for fi, f in enumerate(nonempty):
        u = inv[fi]
        for ij in range(NPOS):
            c = col_of.get((u, ij))
            if c is not None:
                f_sc.append(f)
                ij_sc.append(ij)
                c_sc.append(c)
    return (MA.astype(ml_dtypes.float8_e4m3fn),
            MB.astype(ml_dtypes.float8_e4m3fn), c_pad,
            np.asarray(f_sc), np.asarray(ij_sc), np.asarray(c_sc))


def _build_module(c_pad):
    import concourse.bass as bass
    import concourse.mybir as mybir
    import concourse.tile as tile
    from concourse.masks import make_identity

    f32 = mybir.dt.float32
    fp8 = mybir.dt.float8e4
    i8 = mybir.dt.int8
    Relu = mybir.ActivationFunctionType.Relu
    DR = mybir.MatmulPerfMode.DoubleRow

    nc = bass.Bass("TRN2", target_bir_lowering=False, debug=False,
                   num_devices=N_CORES)

    board_d = nc.dram_tensor("board", [KT, 2 * BPC], fp8,
                             kind="ExternalInput")
    c_half = c_pad // 2
    ma_d = nc.dram_tensor("mmatA", [KT, 2 * c_half], fp8, kind="ExternalInput")
    mb_d = nc.dram_tensor("mmatB", [KT, 2 * c_half], fp8, kind="ExternalInput")
    out_d = nc.dram_tensor("out", [BPC, c_pad], i8, kind="ExternalOutput")

    groups = [(g, min(g + COL_GROUP, c_pad))
              for g in range(0, c_pad, COL_GROUP)]
    nkb = BPC // 128
    PB = 64                    # partition base of the B row-strip copy

    with tile.TileContext(nc) as tc:
        with tc.tile_pool(name="const", bufs=1) as cpool:
            ident = cpool.tile([128, 128], f32)
            make_identity(nc, ident[:])

            # A copies live at partitions 0..41 (PE row strips 0-1), B
            # copies at 64..105 (strips 2-3): consecutive matmuls
            # alternate strips so LDWEIGHTS overlaps the other strip's
            # matmul instead of serializing with it.
            ntile = c_half // COL_TILE
            Mbig = cpool.tile([PB + KT, 2 * c_half], fp8)
            MA3 = Mbig[0:KT, :].rearrange("p (t q n) -> p t q n", t=ntile,
                                          q=2)
            MB3 = Mbig[PB:PB + KT, :].rearrange("p (t q n) -> p t q n",
                                                t=ntile, q=2)
            bbig = cpool.tile([PB + KT, 2 * BPC], fp8)
            bA3 = bbig[0:KT, :].rearrange("p (q n) -> p q n", q=2)
            bB3 = bbig[PB:PB + KT, :].rearrange("p (q n) -> p q n", q=2)

            # ---- phase A: inputs + act-table preload -------------------
            # The board arrives pre-transposed from the host ([KT, 2,
            # 512] fp8, 43KB), so there is no on-device transpose work
            # at all: both PE row-strip copies load straight from HBM.
            with (
                tc.tile_pool(name="prep", bufs=2) as prep,
                tc.tile_pool(name="psW", bufs=1, space="PSUM") as psW,
            ):
                def m_load(eng, which, gi0, gi1):
                    s0, s1 = gi0 * 2 * COL_TILE, gi1 * 2 * COL_TILE
                    if which == 0:
                        eng.dma_start(Mbig[0:KT, s0:s1], ma_d[:, s0:s1])
                    else:
                        eng.dma_start(Mbig[PB:PB + KT, s0:s1],
                                      mb_d[:, s0:s1])

                ngr = len(groups)
                nc.sync.dma_start(bbig[0:KT, :], board_d[:])
                m_load(nc.sync, 0, 0, 1)
                m_load(nc.sync, 1, 0, 1)
                m_load(nc.sync, 0, 2, 3)
                m_load(nc.sync, 0, 3, 5)
                m_load(nc.sync, 0, 5, ngr)
                m_load(nc.sync, 1, 3, 5)
                m_load(nc.sync, 1, 5, ngr)

                # preload the Relu activation table on an independent
                # dummy read of ident, then use ACT's idle early window
                # for the B board copy and the early-group M loads that
                # would otherwise sit deep in SP's serial trigger chain.
                warm8 = prep.tile([1, 2], i8, tag="warm8")
                nc.scalar.activation(warm8[:], ident[0:1, 0:2], Relu)
                m_load(nc.scalar, 0, 1, 2)
                nc.scalar.dma_start(bbig[PB:PB + KT, :], board_d[:])
                m_load(nc.scalar, 1, 1, 2)
                m_load(nc.scalar, 1, 2, 3)

                if WARMUP_MM:
                    wps0 = psW.tile([128, 128], f32, tag="warm")
                    for _ in range(WARMUP_MM):
                        nc.tensor.matmul(wps0[:], ident[:], ident[:],
                                         start=True, stop=True)
                    wrd = prep.tile([32, 1], f32, tag="wrd")
                    nc.vector.tensor_scalar_add(wrd[:], wps0[0:32, 0:1], 0.0)

            # ---- phase B: DoubleRow matmul + relu(int8) + store --------
            # group-outer / kb-inner.  Drains alternate DVE/ACT except
            # the final group (all DVE, so ACT is free to co-trigger the
            # final output wave).  Output chunks: first three groups on
            # the software DGE, middle on SP, final wave as half-height
            # chunks on SP+ACT for a short tail.
            nchunks = len(groups) * nkb
            with (
                tc.tile_pool(name="psM", bufs=4, space="PSUM") as psM,
                tc.tile_pool(name="ostage", bufs=nchunks) as ostage,
            ):
                # ostage has one buffer per chunk: drains never wait on
                # output DMAs and the DMA triggers can lag freely.
                last_g0 = groups[-1][0]
                ci = 0
                for gi, (g0, g1) in enumerate(groups):
                    final = g0 == last_g0
                    for kb in range(nkb):
                        ks = slice(kb * 128, (kb + 1) * 128)
                        pt = psM.tile([128, COL_GROUP], f32, tag="mm")
                        # even 512-tile from strip A, odd from strip B;
                        # their LDWEIGHTS overlap each other's matmuls
                        nc.tensor.matmul(
                            pt[:, 0:COL_TILE], bA3[:, :, ks],
                            MA3[:, gi, :, :],
                            start=True, stop=True, perf_mode=DR,
                            tile_position=(0, 0))
                        nc.tensor.matmul(
                            pt[:, COL_TILE:COL_GROUP], bB3[:, :, ks],
                            MB3[:, gi, :, :],
                            start=True, stop=True, perf_mode=DR,
                            tile_position=(PB, 0))
                        ot = ostage.tile([128, COL_GROUP], i8, tag="ot")
                        # both engines drain one half each in parallel:
                        # half the per-chunk drain latency, so the
                        # 4-deep PSUM pipeline stops hiccuping at pool
                        # wrap-around.  ACT (the busier engine) takes
                        # the A half, which is ready a matmul earlier.
                        nc.scalar.activation(
                            ot[:, 0:COL_TILE], pt[:, 0:COL_TILE], Relu)
                        nc.vector.tensor_scalar_max(
                            ot[:, COL_TILE:COL_GROUP],
                            pt[:, COL_TILE:COL_GROUP], 0.0)
                        ci += 1
                        rows = out_d[kb * 128:(kb + 1) * 128, g0:g1]
                        if final:
                            nc.sync.dma_start(rows[0:64, :], ot[0:64, :])
                            nc.sync.dma_start(rows[64:128, :],
                                              ot[64:128, :])
                        elif gi < 3 or ci % 2 == 0:
                            nc.gpsimd.dma_start(rows, ot[:])
                        else:
                            nc.sync.dma_start(rows, ot[:])
    return nc


def _legalize_multiwait(nc):
    """Split multi-wait instructions for this walrus build.

    The TPB instruction encodings carry exactly one semaphore wait, and
    the walrus codegen here refuses instructions with more ("Too many
    sync wait commands").  Hoist all but one wait onto EventSemaphore
    carrier instructions placed immediately before, on the same engine —
    the sequencer blocks on each carrier first, which is semantically
    identical.
    """
    import concourse.mybir as mybir

    for func in nc.m.functions:
        for blk in func.blocks:
            out = []
            changed = False
            for inst in blk.instructions:
                si = inst.sync_info
                waits = list(si.on_wait) if si is not None and si.on_wait else []
                if len(waits) > 1:
                    for j, w in enumerate(waits[:-1]):
                        carrier = mybir.InstEventSemaphore(
                            name=f"{inst.name}-xw{j}",
                            engine=inst.engine,
                            ins=[], outs=[],
                            sync_info=mybir.SyncInfo(on_wait=[w],
                                                     on_update=[]),
                        )
                        nc.register_instruction(carrier)
                        out.append(carrier)
                    inst.sync_info = mybir.SyncInfo(
                        on_wait=[waits[-1]],
                        on_update=list(si.on_update) if si.on_update else [])
                    changed = True
                out.append(inst)
            if changed:
                blk.instructions = out


_CACHE = {}


def _get_module(c_pad):
    if c_pad not in _CACHE:
        nc = _build_module(c_pad)
        _legalize_multiwait(nc)
        _CACHE[c_pad] = nc
    return _CACHE[c_pad]


def run(board_free, filters, areas, trace=False, **spmd_kwargs):
    from concourse.bass_utils import run_bass_kernel_spmd

    MA, MB, c_pad, f_sc, ij_sc, c_sc = _plan(filters)

    # pre-transpose the boards on the host into the DoubleRow k-tile
    # layout: bT[p, q, b] = [cells, one, one, zero][q*42+p] for board b
    board = np.asarray(board_free, dtype=np.float32).reshape(BATCH, NPOS)
    bT = np.empty((K, BATCH), dtype=np.float32)
    bT[0:NPOS] = board.T
    bT[NPOS] = 1.0
    bT[NPOS + 1] = 1.0
    bT[NPOS + 2:] = 0.0
    bdr = (bT.reshape(2, KT, N_CORES, BPC).transpose(2, 1, 0, 3)
           .reshape(N_CORES, KT, 2 * BPC).astype(ml_dtypes.float8_e4m3fn))
    bdr = np.ascontiguousarray(bdr)

    in_maps = [
        {"board": bdr[c], "mmatA": MA, "mmatB": MB}
        for c in range(N_CORES)
    ]
    nc = _get_module(c_pad)
    res = run_bass_kernel_spmd(nc, in_maps, core_ids=list(range(N_CORES)),
                               trace=trace, **spmd_kwargs)
    raw = np.concatenate([r["out"] for r in res.results], axis=0)

    full = np.zeros((BATCH, NF, NPOS), dtype=np.float32)
    full[:, f_sc, ij_sc] = raw[:, c_sc]
    return full.reshape(BATCH, NF, 9, 9), res


def kernel(board_free, filters, areas):
    out, _ = run(board_free, filters, areas)
    return out
